# revision 1
# baseline (speedup 1.0000x reference)
"""DPOT2D layer (AFNO-style) Trainium2 kernel.

out = x + irfft2_pad(blockMLP(trunc64(rfft2(x))))   (ortho norm)

Sharding: tensor-parallel over the 8 block-diagonal channel groups — core n
gets channels [n*64, (n+1)*64) and its block's MLP weights. Blocks never mix,
so there is zero cross-core communication.

Per core, every FFT stage is a DFT matmul on the TensorEngine (bf16 operands,
fp32 PSUM accumulation), with PE-transpose corner turns between stages:

  A:  U[k1s,(w,c)]   = F_h^T  @ x          (contract h, 2x128 K-chunks)
  t1: V[w,(c,k1s)]   = corner turn of U
  B:  Y[k2s,(c,k1)]  = DFT_w on complex U  (re/im column accumulation)
  t2: Yt[c,(k1,k2s)] = corner turn of Y
  L1: o1 = gelu(W1 Y + b1)                 (K=64, re/im col accumulation)
  L2: O2[(o2r|o2i),(k1,k2)] = W2 o1 + b2   (K=128)
  t3: R[k2,(k1,o2s)] = corner turn of O2
  iW: G[w,(k1,c')]   = hermitian irfft_w matmuls (re/im col accumulation)
  t4: Ght[k1s,(w,c')]= corner turn of G (re/im interleaved -> k1-stack)
  iH: x'[h,(w,c')]   = E_h^T @ Ght, + residual x (fp32), DMA out
"""

import numpy as np
import ml_dtypes

import concourse.bass as bass
import concourse.mybir as mybir
from concourse import bacc
from concourse import masks
from concourse.tile import TileContext
from concourse.bass_utils import run_bass_kernel_spmd

B = 2
H = 256
W = 256
C = 512
NB = 8
BS = 64          # channels per block (= per core)
KEEP = 64        # kept modes per spatial dim
HID = 128
P = 128

BF16 = mybir.dt.bfloat16
F32 = mybir.dt.float32
AF = mybir.ActivationFunctionType

_CACHED_NC = None


def _host_consts():
    """DFT matrices shared by all cores (fp32 -> bf16)."""
    h = np.arange(H, dtype=np.float64)[:, None]
    k = np.arange(KEEP, dtype=np.float64)[None, :]
    th = 2.0 * np.pi * h * k / H
    F = np.concatenate([np.cos(th), -np.sin(th)], axis=1) / 16.0      # (256,128)
    Fwre, Fwim = F[:, :KEEP], F[:, KEEP:]
    lb_re = np.concatenate([Fwre, Fwim], axis=1)                      # (256,128)
    lb_im = np.concatenate([-Fwim, Fwre], axis=1)
    alpha = np.where(np.arange(KEEP) == 0, 1.0, 2.0)
    k2 = np.arange(KEEP, dtype=np.float64)[:, None]
    wv = np.arange(W, dtype=np.float64)[None, :]
    tw = 2.0 * np.pi * k2 * wv / W
    Ca = alpha[:, None] * np.cos(tw) / 16.0                           # (64,256)
    Sa = alpha[:, None] * np.sin(tw) / 16.0
    k1 = np.arange(KEEP, dtype=np.float64)[:, None]
    hv = np.arange(H, dtype=np.float64)[None, :]
    tih = 2.0 * np.pi * k1 * hv / H
    Ehc = np.cos(tih) / 16.0                                          # (64,256)
    Ehs = np.sin(tih) / 16.0
    lih_full = np.concatenate([Ehc, -Ehs], axis=0)                    # (128,256)

    bf = ml_dtypes.bfloat16
    ffwd = np.stack([F[0:128], F[128:256]]).astype(bf)                # (2,128,128)
    lbw = np.stack([
        np.stack([lb_re[0:128], lb_im[0:128]]),
        np.stack([lb_re[128:256], lb_im[128:256]]),
    ]).astype(bf)                                                     # (2,2,128,128)
    liw = np.stack([
        np.stack([np.stack([Ca[:, 0:128], -Sa[:, 0:128]]),
                  np.stack([Sa[:, 0:128], Ca[:, 0:128]])]),
        np.stack([np.stack([Ca[:, 128:256], -Sa[:, 128:256]]),
                  np.stack([Sa[:, 128:256], Ca[:, 128:256]])]),
    ]).astype(bf)                                                     # (2,2,2,64,128)
    lih = np.stack([lih_full[:, 0:128], lih_full[:, 128:256]]).astype(bf)  # (2,128,128)
    return ffwd, lbw, liw, lih


def _build_nc(loop_iters=0, probe=None):
    """loop_iters>0 wraps the whole per-batch pipeline in an on-device
    For_i repeat loop — used only by the timing harness to amortize the
    ~80ms axon dispatch overhead out of the measurement.
    probe: None | 'dma' (DMAs only) | 'compute' (no input/residual DMAs)."""
    nc = bacc.Bacc()

    xbf = nc.declare_dram_parameter("xbf", [B, H, W, BS], BF16, isOutput=False)
    xf32 = nc.declare_dram_parameter("xf32", [B, H, W, BS], F32, isOutput=False)
    ffwd_d = nc.declare_dram_parameter("ffwd", [2, P, P], BF16, isOutput=False)
    lbw_d = nc.declare_dram_parameter("lbw", [2, 2, P, P], BF16, isOutput=False)
    m1_d = nc.declare_dram_parameter("m1", [2, 2, BS, HID], BF16, isOutput=False)
    m2_d = nc.declare_dram_parameter("m2", [2, HID, P], BF16, isOutput=False)
    b1s_d = nc.declare_dram_parameter("b1s", [2, HID, 1], F32, isOutput=False)
    b2s_d = nc.declare_dram_parameter("b2s", [P, 1], F32, isOutput=False)
    liw_d = nc.declare_dram_parameter("liw", [2, 2, 2, KEEP, P], BF16, isOutput=False)
    lih_d = nc.declare_dram_parameter("lih", [2, P, P], BF16, isOutput=False)
    out = nc.declare_dram_parameter("out", [B, H, W, BS], F32, isOutput=True)

    with TileContext(nc) as tc:
        consts = tc.alloc_tile_pool(name="consts", bufs=1)
        ident = consts.tile([P, P], BF16, name="ident")
        masks.make_identity(nc, ident[:])

        def const2d(name, dram_ap, shape, dtype=BF16):
            t = consts.tile(shape, dtype, name=name)
            nc.sync.dma_start(out=t[:], in_=dram_ap)
            return t

        FW = [const2d(f"fw{hh}", ffwd_d[hh], [P, P]) for hh in range(2)]
        LBW = [[const2d(f"lbw{wh}{s}", lbw_d[wh, s], [P, P]) for s in range(2)]
               for wh in range(2)]
        M1 = [[const2d(f"m1_{j}{s}", m1_d[j, s], [BS, HID]) for s in range(2)]
              for j in range(2)]
        M2 = [const2d(f"m2_{s}", m2_d[s], [HID, P]) for s in range(2)]
        LIW = [[[const2d(f"liw{wh}{j}{s}", liw_d[wh, j, s], [KEEP, P])
                 for s in range(2)] for j in range(2)] for wh in range(2)]
        LIH = [const2d(f"lih{hc}", lih_d[hc], [P, P]) for hc in range(2)]
        b1s_t = [const2d(f"b1s{j}", b1s_d[j], [HID, 1], F32) for j in range(2)]
        b2s_t = const2d("b2s", b2s_d[:], [P, 1], F32)

        # copy-engine rotation (PSUM-capable engines only: DVE + ACT)
        cp_cnt = [0]

        def cp(dst, src):
            i = cp_cnt[0] % 2
            cp_cnt[0] += 1
            if i == 0:
                nc.vector.tensor_copy(out=dst, in_=src)
            else:
                nc.scalar.activation(out=dst, in_=src, func=AF.Copy)

        # Tag-sharing across stage lifetimes keeps SBUF within budget:
        #   tagA/tagB: U[wh] -> G[wh]   tagC/tagD: V[wh] -> Ght[wh]
        #   tagE: Y -> R                tagF: Yt -> O2
        sb = tc.alloc_tile_pool(name="sb", bufs=1)
        xin = tc.alloc_tile_pool(name="xin", bufs=2)
        xres_p = tc.alloc_tile_pool(name="xres", bufs=2)
        outp = tc.alloc_tile_pool(name="outp", bufs=2)
        pmm = tc.alloc_tile_pool(name="pmm", bufs=4, space="PSUM")
        ptp = tc.alloc_tile_pool(name="ptp", bufs=4, space="PSUM")

        import contextlib
        loop_ctx = tc.For_i(0, loop_iters, 1) if loop_iters else contextlib.nullcontext()
        with loop_ctx:
            if probe == "dma":
                _emit_dma_probe(nc, tc, locals())
            else:
                _emit_body(nc, tc, locals(), skip_dma=(probe == "compute"))
        ptp.release()
        pmm.release()
        outp.release()
        xres_p.release()
        xin.release()
        sb.release()
        consts.release()
    nc.compile()
    return nc


def _emit_dma_probe(nc, tc, env):
    """Same DMA traffic as the real kernel (x-in bf16, x-res f32, out f32),
    no compute: out tiles are fed straight from the xres tiles."""
    xbf = env["xbf"]; xf32 = env["xf32"]; out = env["out"]
    xin = env["xin"]; xres_p = env["xres_p"]
    for b in range(B):
        for wc in range(8):
            for hh in range(2):
                t = xin.tile([P, 32, BS], BF16, tag=f"xin{hh}",
                             name=f"pxin{hh}_{b}_{wc}")
                nc.sync.dma_start(
                    out=t[:],
                    in_=xbf[b, hh * P:(hh + 1) * P, wc * 32:(wc + 1) * 32, :])
        for hc in range(2):
            for q8 in range(8):
                xr = xres_p.tile([P, 32, BS], F32, tag="xres",
                                 name=f"pxr_{b}_{hc}_{q8}")
                nc.sync.dma_start(
                    out=xr[:],
                    in_=xf32[b, hc * P:(hc + 1) * P, q8 * 32:(q8 + 1) * 32, :])
                nc.sync.dma_start(
                    out=out[b, hc * P:(hc + 1) * P, q8 * 32:(q8 + 1) * 32, :],
                    in_=xr[:])


def _emit_body(nc, tc, env, skip_dma=False):
    xbf = env["xbf"]; xf32 = env["xf32"]; out = env["out"]
    FW = env["FW"]; LBW = env["LBW"]; M1 = env["M1"]; M2 = env["M2"]
    LIW = env["LIW"]; LIH = env["LIH"]; b1s_t = env["b1s_t"]; b2s_t = env["b2s_t"]
    ident = env["ident"]; cp = env["cp"]; cp_cnt = env["cp_cnt"]
    sb = env["sb"]; xin = env["xin"]; xres_p = env["xres_p"]; outp = env["outp"]
    pmm = env["pmm"]; ptp = env["ptp"]

    if True:
        for b in range(B):
            # ---------------- stage A: U[wh] (128=k1s, (w 128, c 64)) ----------
            U = [sb.tile([P, 128, BS], BF16, tag=f"tagAB{wh}", name=f"U{wh}_{b}")
                 for wh in range(2)]
            for wc in range(8):          # w chunks of 32
                xt = []
                for hh in range(2):
                    t = xin.tile([P, 32, BS], BF16, tag=f"xin{hh}",
                                 name=f"xin{hh}_{b}_{wc}")
                    if not skip_dma:
                        nc.sync.dma_start(
                            out=t[:],
                            in_=xbf[b, hh * P:(hh + 1) * P, wc * 32:(wc + 1) * 32, :])
                    else:
                        nc.sync.dma_start(
                            out=t[0:1, 0:1, :],
                            in_=xbf[b, 0:1, 0:1, :])
                    xt.append(t)
                for nn in range(4):      # N=512 pieces (8 w each)
                    ps = pmm.tile([P, 8, BS], F32, tag="mm", name=f"psA_{b}_{wc}_{nn}")
                    nc.tensor.matmul(ps[:], FW[0], xt[0][:, nn * 8:(nn + 1) * 8, :],
                                     start=True, stop=False)
                    nc.tensor.matmul(ps[:], FW[1], xt[1][:, nn * 8:(nn + 1) * 8, :],
                                     start=False, stop=True)
                    wg = wc * 4 + nn     # global 8-w group index (0..31)
                    cp(U[wg // 16][:, (wg % 16) * 8:(wg % 16) * 8 + 8, :], ps[:])

            # ---------------- turn1: V[wh] (128=w, (c 64, k1s 128)) ------------
            V = [sb.tile([P, BS, P], BF16, tag=f"tagCD{wh}", name=f"V{wh}_{b}")
                 for wh in range(2)]
            for wh in range(2):
                for c in range(BS):
                    pt = ptp.tile([P, P], BF16, tag="tp", name=f"t1_{b}_{wh}_{c}")
                    nc.tensor.transpose(pt[:], U[wh][:, :, c], ident[:])
                    cp(V[wh][:, c, :], pt[:])

            # ---------------- stage B: Y (128=k2s, (c 64, k1 64)) --------------
            Y = sb.tile([P, BS, KEEP], BF16, tag="tagE", name=f"Y_{b}")
            for nn in range(8):          # 8 c per chunk -> N=512
                ps = pmm.tile([P, 8, KEEP], F32, tag="mm", name=f"psB_{b}_{nn}")
                first = True
                for wh in range(2):
                    for s in range(2):   # 0: re cols (k1s 0:64), 1: im cols
                        rhs = V[wh][:, nn * 8:(nn + 1) * 8, s * KEEP:(s + 1) * KEEP]
                        nc.tensor.matmul(ps[:], LBW[wh][s], rhs,
                                         start=first, stop=(wh == 1 and s == 1))
                        first = False
                cp(Y[:, nn * 8:(nn + 1) * 8, :], ps[:])

            # ---------------- turn2: Yt (64=c, (k1 64, k2s 128)) ---------------
            Yt = sb.tile([BS, KEEP, P], BF16, tag="tagF", name=f"Yt_{b}")
            for k1 in range(KEEP):
                pt = ptp.tile([BS, P], BF16, tag="tp", name=f"t2_{b}_{k1}")
                nc.tensor.transpose(pt[:], Y[:, :, k1], ident[:])
                cp(Yt[:, k1, :], pt[:])

            # ---------------- MLP L1 (K=64) + gelu -----------------------------
            o1 = [sb.tile([HID, KEEP, KEEP], BF16, tag=f"o1_{j}", name=f"o1_{j}_{b}")
                  for j in range(2)]
            for j in range(2):
                for nn in range(8):      # 8 k1 per chunk -> N=512
                    ps = pmm.tile([HID, 8, KEEP], F32, tag="mm",
                                  name=f"ps1_{b}_{j}_{nn}")
                    nc.tensor.matmul(
                        ps[:], M1[j][0],
                        Yt[:, nn * 8:(nn + 1) * 8, 0:KEEP], start=True, stop=False)
                    nc.tensor.matmul(
                        ps[:], M1[j][1],
                        Yt[:, nn * 8:(nn + 1) * 8, KEEP:P], start=False, stop=True)
                    nc.scalar.activation(out=o1[j][:, nn * 8:(nn + 1) * 8, :],
                                         in_=ps[:], func=AF.Gelu, bias=b1s_t[j][:])

            # ---------------- MLP L2 (K=128) + bias ----------------------------
            O2 = sb.tile([P, KEEP, KEEP], BF16, tag="tagF", name=f"O2_{b}")
            for nn in range(8):
                ps = pmm.tile([P, 8, KEEP], F32, tag="mm", name=f"ps2_{b}_{nn}")
                nc.tensor.matmul(ps[:], M2[0], o1[0][:, nn * 8:(nn + 1) * 8, :],
                                 start=True, stop=False)
                nc.tensor.matmul(ps[:], M2[1], o1[1][:, nn * 8:(nn + 1) * 8, :],
                                 start=False, stop=True)
                if cp_cnt[0] % 2 == 0:
                    nc.vector.tensor_scalar_add(
                        out=O2[:, nn * 8:(nn + 1) * 8, :], in0=ps[:], scalar1=b2s_t[:])
                else:
                    nc.scalar.activation(out=O2[:, nn * 8:(nn + 1) * 8, :],
                                         in_=ps[:], func=AF.Identity, bias=b2s_t[:])
                cp_cnt[0] += 1

            # ---------------- turn3: R (64=k2, (k1 64, o2s 128)) ---------------
            R = sb.tile([KEEP, KEEP, P], BF16, tag="tagE", name=f"R_{b}")
            for k1 in range(KEEP):
                pt = ptp.tile([KEEP, P], BF16, tag="tp", name=f"t3_{b}_{k1}")
                nc.tensor.transpose(pt[:], O2[:, k1, :], ident[:])
                cp(R[:, k1, :], pt[:])

            # ---------------- invW: G[wh] (128=w, (j 2, k1 64, c' 64)) ---------
            G = [sb.tile([P, 2, KEEP, BS], BF16, tag=f"tagAB{wh}", name=f"G{wh}_{b}")
                 for wh in range(2)]
            for wh in range(2):
                for j in range(2):       # 0: Gre, 1: Gim
                    for nn in range(8):  # 8 k1 per chunk
                        ps = pmm.tile([P, 8, BS], F32, tag="mm",
                                      name=f"psW_{b}_{wh}_{j}_{nn}")
                        nc.tensor.matmul(
                            ps[:], LIW[wh][j][0],
                            R[:, nn * 8:(nn + 1) * 8, 0:KEEP],
                            start=True, stop=False)
                        nc.tensor.matmul(
                            ps[:], LIW[wh][j][1],
                            R[:, nn * 8:(nn + 1) * 8, KEEP:P],
                            start=False, stop=True)
                        cp(G[wh][:, j, nn * 8:(nn + 1) * 8, :], ps[:])

            # ---------------- turn4: Ght (128=k1s, (w 256, c' 64)) -------------
            Ght = [sb.tile([P, P, BS], BF16, tag=f"tagCD{wh}", name=f"Ght{wh}_{b}")
                   for wh in range(2)]
            for wh in range(2):
                for c in range(BS):
                    pt = ptp.tile([P, P], BF16, tag="tp", name=f"t4_{b}_{wh}_{c}")
                    # free slice (j 2, k1 64) -> out partitions [k1re | k1im]
                    nc.tensor.transpose(pt[:], G[wh][:, :, :, c], ident[:])
                    cp(Ght[wh][:, :, c], pt[:])

            # ---------------- invH + residual + store --------------------------
            for hc in range(2):
                for q8 in range(8):      # groups of 32 w
                    xr = xres_p.tile([P, 32, BS], F32, tag="xres",
                                     name=f"xr_{b}_{hc}_{q8}")
                    if not skip_dma:
                        nc.sync.dma_start(
                            out=xr[:],
                            in_=xf32[b, hc * P:(hc + 1) * P, q8 * 32:(q8 + 1) * 32, :])
                    else:
                        nc.sync.dma_start(
                            out=xr[0:1, 0:1, :],
                            in_=xf32[b, 0:1, 0:1, :])
                    ot = outp.tile([P, 32, BS], F32, tag="ot",
                                   name=f"ot_{b}_{hc}_{q8}")
                    for nn in range(4):  # N=512 pieces (8 w each)
                        wg = q8 * 4 + nn         # global 8-w group (0..31)
                        ps = pmm.tile([P, 8, BS], F32, tag="mm",
                                      name=f"psH_{b}_{hc}_{wg}")
                        nc.tensor.matmul(
                            ps[:], LIH[hc],
                            Ght[wg // 16][:, (wg % 16) * 8:(wg % 16) * 8 + 8, :],
                            start=True, stop=True)
                        nc.vector.tensor_add(
                            out=ot[:, nn * 8:(nn + 1) * 8, :], in0=ps[:],
                            in1=xr[:, nn * 8:(nn + 1) * 8, :])
                    nc.sync.dma_start(
                        out=out[b, hc * P:(hc + 1) * P,
                                q8 * 32:(q8 + 1) * 32, :],
                        in_=ot[:])


def _prepare_in_maps(x, w1, b1, w2, b2):
    bf = ml_dtypes.bfloat16
    ffwd, lbw, liw, lih = _host_consts()
    x = np.asarray(x, dtype=np.float32)

    in_maps = []
    for n in range(NB):
        xs = np.ascontiguousarray(x[..., n * BS:(n + 1) * BS])
        w1n = np.asarray(w1[:, n], dtype=np.float32)   # (2,64,128)
        w2n = np.asarray(w2[:, n], dtype=np.float32)   # (2,128,64)
        b1n = np.asarray(b1[:, n], dtype=np.float32)   # (2,128)
        b2n = np.asarray(b2[:, n], dtype=np.float32)   # (2,64)
        m1 = np.stack([
            np.stack([w1n[0], -w1n[1]]),
            np.stack([w1n[1], w1n[0]]),
        ]).astype(bf)                                   # (2,2,64,128)
        m2 = np.stack([
            np.concatenate([w2n[0], w2n[1]], axis=1),
            np.concatenate([-w2n[1], w2n[0]], axis=1),
        ]).astype(bf)                                   # (2,128,128)
        in_maps.append({
            "xbf": xs.astype(bf),
            "xf32": xs,
            "ffwd": ffwd,
            "lbw": lbw,
            "m1": m1,
            "m2": m2,
            "b1s": b1n[:, :, None].copy(),
            "b2s": np.concatenate([b2n[0], b2n[1]])[:, None].copy(),
            "liw": liw,
            "lih": lih,
        })

    return in_maps


def kernel(x, w1, b1, w2, b2):
    global _CACHED_NC
    if _CACHED_NC is None:
        _CACHED_NC = _build_nc()
    nc = _CACHED_NC
    in_maps = _prepare_in_maps(x, w1, b1, w2, b2)
    res = run_bass_kernel_spmd(nc, in_maps, list(range(NB)))
    return np.concatenate([res.results[i]["out"] for i in range(NB)], axis=-1)



# revision 2
# speedup vs baseline: 2.3225x; 2.3225x over previous
"""DPOT2D layer (AFNO-style) Trainium2 kernel.

out = x + irfft2_pad(blockMLP(trunc64(rfft2(x))))   (ortho norm)

Sharding: tensor-parallel over the 8 block-diagonal channel groups — core n
gets channels [n*64, (n+1)*64) and its block's MLP weights. Blocks never mix,
so there is zero cross-core communication.

Per core, every FFT stage is a DFT matmul on the TensorEngine (bf16 operands,
fp32 PSUM accumulation), with PE-transpose corner turns between stages:

  A:  U[k1s,(w,c)]   = F_h^T  @ x          (contract h, 2x128 K-chunks)
  t1: V[w,(c,k1s)]   = corner turn of U
  B:  Y[k2s,(c,k1)]  = DFT_w on complex U  (re/im column accumulation)
  t2: Yt[c,(k1,k2s)] = corner turn of Y
  L1: o1 = gelu(W1 Y + b1)                 (K=64, re/im col accumulation)
  L2: O2[(o2r|o2i),(k1,k2)] = W2 o1 + b2   (K=128)
  t3: R[k2,(k1,o2s)] = corner turn of O2
  iW: G[w,(j,k1,c')] = hermitian irfft_w matmuls (re/im col accumulation)
  t4: Ght[k1s,(w,c')]= corner turn of G (re/im interleaved -> k1-stack)
  iH: x'[h,(w,c')]   = E_h^T @ Ght, + residual x (bf16), DMA out

I/O precision: x is loaded once as bf16 (8 MB/core) and kept resident for the
residual add; the output is stored as bf16 (8 MB/core) and upcast to f32 on
the host. The spectral correction is ~3e-5 of the output norm, so total
rel-err is dominated by bf16 rounding of x (~2e-3), well inside the 2e-2 gate.
"""

import numpy as np
import ml_dtypes

import concourse.bass as bass
import concourse.mybir as mybir
from concourse import bacc
from concourse import masks
from concourse.tile import TileContext
from concourse.bass_utils import run_bass_kernel_spmd

B = 2
H = 256
W = 256
C = 512
NB = 8
BS = 64          # channels per block (= per core)
KEEP = 64        # kept modes per spatial dim
HID = 128
P = 128

BF16 = mybir.dt.bfloat16
F32 = mybir.dt.float32
AF = mybir.ActivationFunctionType

_CACHED_NC = None


def _host_consts():
    """DFT matrices shared by all cores (fp32 -> bf16)."""
    h = np.arange(H, dtype=np.float64)[:, None]
    k = np.arange(KEEP, dtype=np.float64)[None, :]
    th = 2.0 * np.pi * h * k / H
    F = np.concatenate([np.cos(th), -np.sin(th)], axis=1) / 16.0      # (256,128)
    Fwre, Fwim = F[:, :KEEP], F[:, KEEP:]
    lb_re = np.concatenate([Fwre, Fwim], axis=1)                      # (256,128)
    lb_im = np.concatenate([-Fwim, Fwre], axis=1)
    alpha = np.where(np.arange(KEEP) == 0, 1.0, 2.0)
    k2 = np.arange(KEEP, dtype=np.float64)[:, None]
    wv = np.arange(W, dtype=np.float64)[None, :]
    tw = 2.0 * np.pi * k2 * wv / W
    Ca = alpha[:, None] * np.cos(tw) / 16.0                           # (64,256)
    Sa = alpha[:, None] * np.sin(tw) / 16.0
    k1 = np.arange(KEEP, dtype=np.float64)[:, None]
    hv = np.arange(H, dtype=np.float64)[None, :]
    tih = 2.0 * np.pi * k1 * hv / H
    Ehc = np.cos(tih) / 16.0                                          # (64,256)
    Ehs = np.sin(tih) / 16.0
    lih_full = np.concatenate([Ehc, -Ehs], axis=0)                    # (128,256)

    bf = ml_dtypes.bfloat16
    ffwd = np.stack([F[0:128], F[128:256]]).astype(bf)                # (2,128,128)
    lbw = np.stack([
        np.stack([lb_re[0:128], lb_im[0:128]]),
        np.stack([lb_re[128:256], lb_im[128:256]]),
    ]).astype(bf)                                                     # (2,2,128,128)
    liw = np.stack([
        np.stack([np.stack([Ca[:, 0:128], -Sa[:, 0:128]]),
                  np.stack([Sa[:, 0:128], Ca[:, 0:128]])]),
        np.stack([np.stack([Ca[:, 128:256], -Sa[:, 128:256]]),
                  np.stack([Sa[:, 128:256], Ca[:, 128:256]])]),
    ]).astype(bf)                                                     # (2,2,2,64,128)
    lih = np.stack([lih_full[:, 0:128], lih_full[:, 128:256]]).astype(bf)  # (2,128,128)
    return ffwd, lbw, liw, lih


def _build_nc(loop_iters=0, probe=None):
    """loop_iters>0 wraps the whole per-batch pipeline in an on-device
    For_i repeat loop — used only by the timing harness to amortize the
    ~80ms axon dispatch overhead out of the measurement.
    probe: None | 'dma' (DMAs only) | 'compute' (token input/output DMAs)."""
    nc = bacc.Bacc()

    xbf = nc.declare_dram_parameter("xbf", [B, H, W, BS], BF16, isOutput=False)
    ffwd_d = nc.declare_dram_parameter("ffwd", [2, P, P], BF16, isOutput=False)
    lbw_d = nc.declare_dram_parameter("lbw", [2, 2, P, P], BF16, isOutput=False)
    m1_d = nc.declare_dram_parameter("m1", [2, 2, BS, HID], BF16, isOutput=False)
    m2_d = nc.declare_dram_parameter("m2", [2, HID, P], BF16, isOutput=False)
    b1s_d = nc.declare_dram_parameter("b1s", [2, HID, 1], F32, isOutput=False)
    b2s_d = nc.declare_dram_parameter("b2s", [P, 1], F32, isOutput=False)
    liw_d = nc.declare_dram_parameter("liw", [2, 2, 2, KEEP, P], BF16, isOutput=False)
    lih_d = nc.declare_dram_parameter("lih", [2, P, P], BF16, isOutput=False)
    out = nc.declare_dram_parameter("out", [B, H, W, BS], BF16, isOutput=True)

    with TileContext(nc) as tc:
        consts = tc.alloc_tile_pool(name="consts", bufs=1)
        ident = consts.tile([P, P], BF16, name="ident")
        masks.make_identity(nc, ident[:])

        def const2d(name, dram_ap, shape, dtype=BF16):
            t = consts.tile(shape, dtype, name=name)
            nc.sync.dma_start(out=t[:], in_=dram_ap)
            return t

        FW = [const2d(f"fw{hh}", ffwd_d[hh], [P, P]) for hh in range(2)]
        LBW = [[const2d(f"lbw{wh}{s}", lbw_d[wh, s], [P, P]) for s in range(2)]
               for wh in range(2)]
        M1 = [[const2d(f"m1_{j}{s}", m1_d[j, s], [BS, HID]) for s in range(2)]
              for j in range(2)]
        M2 = [const2d(f"m2_{s}", m2_d[s], [HID, P]) for s in range(2)]
        LIW = [[[const2d(f"liw{wh}{j}{s}", liw_d[wh, j, s], [KEEP, P])
                 for s in range(2)] for j in range(2)] for wh in range(2)]
        LIH = [const2d(f"lih{hc}", lih_d[hc], [P, P]) for hc in range(2)]
        b1s_t = [const2d(f"b1s{j}", b1s_d[j], [HID, 1], F32) for j in range(2)]
        b2s_t = const2d("b2s", b2s_d[:], [P, 1], F32)

        # copy-engine rotation (PSUM-capable engines only: DVE + ACT),
        # weighted toward DVE (ACT copies are slower on HW and ACT also
        # carries the gelu evacuations).
        cp_cnt = [0]
        CP_PATTERN = ("v", "v", "a")

        def cp(dst, src):
            kind = CP_PATTERN[cp_cnt[0] % len(CP_PATTERN)]
            cp_cnt[0] += 1
            if kind == "v":
                nc.vector.tensor_copy(out=dst, in_=src)
            else:
                nc.scalar.activation(out=dst, in_=src, func=AF.Copy)

        # Tag-sharing across stage lifetimes keeps SBUF within budget:
        #   tagA/tagB: U[wh] -> G[wh]   tagC/tagD: V[wh] -> Ght[wh]
        #   tagE: Y -> R                tagF: Yt -> O2
        sb = tc.alloc_tile_pool(name="sb", bufs=1)
        xin = tc.alloc_tile_pool(name="xin", bufs=1)
        outp = tc.alloc_tile_pool(name="outp", bufs=2)
        pmm = tc.alloc_tile_pool(name="pmm", bufs=4, space="PSUM")
        ptp = tc.alloc_tile_pool(name="ptp", bufs=4, space="PSUM")

        import contextlib
        loop_ctx = tc.For_i(0, loop_iters, 1) if loop_iters else contextlib.nullcontext()
        with loop_ctx:
            _emit_body(nc, tc, locals(), probe=probe)
        ptp.release()
        pmm.release()
        outp.release()
        xin.release()
        sb.release()
        consts.release()
    nc.compile()
    return nc


def _emit_body(nc, tc, env, probe=None):
    xbf = env["xbf"]; out = env["out"]
    FW = env["FW"]; LBW = env["LBW"]; M1 = env["M1"]; M2 = env["M2"]
    LIW = env["LIW"]; LIH = env["LIH"]; b1s_t = env["b1s_t"]; b2s_t = env["b2s_t"]
    ident = env["ident"]; cp = env["cp"]; cp_cnt = env["cp_cnt"]
    sb = env["sb"]; xin = env["xin"]; outp = env["outp"]
    pmm = env["pmm"]; ptp = env["ptp"]

    dma_only = probe == "dma"
    no_io = probe == "compute"

    for b in range(B):
        # -------- input loads: 8 x 1MB [128h, 64w, 64c], resident all batch
        xt = [[None] * 4 for _ in range(2)]
        for hh in range(2):
            for wq in range(4):
                t = xin.tile([P, 64, BS], BF16, tag=f"xin{hh}{wq}",
                             name=f"xin{hh}{wq}_{b}")
                if no_io:
                    nc.sync.dma_start(out=t[0:1, 0:1, :], in_=xbf[b, 0:1, 0:1, :])
                else:
                    nc.sync.dma_start(
                        out=t[:],
                        in_=xbf[b, hh * P:(hh + 1) * P, wq * 64:(wq + 1) * 64, :])
                xt[hh][wq] = t
        if dma_only:
            # stores fed straight from the input tiles: same DMA traffic,
            # no compute.
            for hc in range(2):
                for wq in range(4):
                    nc.scalar.dma_start(
                        out=out[b, hc * P:(hc + 1) * P, wq * 64:(wq + 1) * 64, :],
                        in_=xt[hc][wq][:])
            continue

        # ---------------- stage A: U[wh] (128=k1s, (w 128, c 64)) ----------
        U = [sb.tile([P, 128, BS], BF16, tag=f"tagAB{wh}", name=f"U{wh}_{b}")
             for wh in range(2)]
        for wq in range(4):          # w chunks of 64
            for nn in range(8):      # 8 w each -> N=512
                ps = pmm.tile([P, 8, BS], F32, tag="mm", name=f"psA_{b}_{wq}_{nn}")
                nc.tensor.matmul(ps[:], FW[0], xt[0][wq][:, nn * 8:(nn + 1) * 8, :],
                                 start=True, stop=False)
                nc.tensor.matmul(ps[:], FW[1], xt[1][wq][:, nn * 8:(nn + 1) * 8, :],
                                 start=False, stop=True)
                wg = wq * 8 + nn     # global 8-w group index (0..31)
                cp(U[wg // 16][:, (wg % 16) * 8:(wg % 16) * 8 + 8, :], ps[:])

        # ---------------- turn1: V[wh] (128=w, (c 64, k1s 128)) ------------
        # 4 transposes share one PSUM tile -> single 512-col evacuation.
        V = [sb.tile([P, BS, P], BF16, tag=f"tagCD{wh}", name=f"V{wh}_{b}")
             for wh in range(2)]
        for wh in range(2):
            for cg in range(16):     # groups of 4 channels
                pt = ptp.tile([P, 4, P], BF16, tag="tp", name=f"t1_{b}_{wh}_{cg}")
                for i in range(4):
                    nc.tensor.transpose(pt[:, i, :], U[wh][:, :, cg * 4 + i],
                                        ident[:])
                cp(V[wh][:, cg * 4:cg * 4 + 4, :], pt[:])

        # ---------------- stage B: Y (128=k2s, (c 64, k1 64)) --------------
        Y = sb.tile([P, BS, KEEP], BF16, tag="tagE", name=f"Y_{b}")
        for nn in range(8):          # 8 c per chunk -> N=512
            ps = pmm.tile([P, 8, KEEP], F32, tag="mm", name=f"psB_{b}_{nn}")
            first = True
            for wh in range(2):
                for s in range(2):   # 0: re cols (k1s 0:64), 1: im cols
                    rhs = V[wh][:, nn * 8:(nn + 1) * 8, s * KEEP:(s + 1) * KEEP]
                    nc.tensor.matmul(ps[:], LBW[wh][s], rhs,
                                     start=first, stop=(wh == 1 and s == 1))
                    first = False
            cp(Y[:, nn * 8:(nn + 1) * 8, :], ps[:])

        # ---------------- turn2: Yt (64=c, (k1 64, k2s 128)) ---------------
        Yt = sb.tile([BS, KEEP, P], BF16, tag="tagF", name=f"Yt_{b}")
        for kg in range(16):         # groups of 4 k1
            pt = ptp.tile([BS, 4, P], BF16, tag="tp", name=f"t2_{b}_{kg}")
            for i in range(4):
                nc.tensor.transpose(pt[:, i, :], Y[:, :, kg * 4 + i], ident[:])
            cp(Yt[:, kg * 4:kg * 4 + 4, :], pt[:])

        # ---------------- MLP L1 (K=64) + gelu -----------------------------
        o1 = [sb.tile([HID, KEEP, KEEP], BF16, tag=f"o1_{j}", name=f"o1_{j}_{b}")
              for j in range(2)]
        for j in range(2):
            for nn in range(8):      # 8 k1 per chunk -> N=512
                ps = pmm.tile([HID, 8, KEEP], F32, tag="mm",
                              name=f"ps1_{b}_{j}_{nn}")
                nc.tensor.matmul(
                    ps[:], M1[j][0],
                    Yt[:, nn * 8:(nn + 1) * 8, 0:KEEP], start=True, stop=False)
                nc.tensor.matmul(
                    ps[:], M1[j][1],
                    Yt[:, nn * 8:(nn + 1) * 8, KEEP:P], start=False, stop=True)
                nc.scalar.activation(out=o1[j][:, nn * 8:(nn + 1) * 8, :],
                                     in_=ps[:], func=AF.Gelu, bias=b1s_t[j][:])

        # ---------------- MLP L2 (K=128) + bias ----------------------------
        O2 = sb.tile([P, KEEP, KEEP], BF16, tag="tagF", name=f"O2_{b}")
        for nn in range(8):
            ps = pmm.tile([P, 8, KEEP], F32, tag="mm", name=f"ps2_{b}_{nn}")
            nc.tensor.matmul(ps[:], M2[0], o1[0][:, nn * 8:(nn + 1) * 8, :],
                             start=True, stop=False)
            nc.tensor.matmul(ps[:], M2[1], o1[1][:, nn * 8:(nn + 1) * 8, :],
                             start=False, stop=True)
            if cp_cnt[0] % 3 != 2:
                nc.vector.tensor_scalar_add(
                    out=O2[:, nn * 8:(nn + 1) * 8, :], in0=ps[:], scalar1=b2s_t[:])
            else:
                nc.scalar.activation(out=O2[:, nn * 8:(nn + 1) * 8, :],
                                     in_=ps[:], func=AF.Identity, bias=b2s_t[:])
            cp_cnt[0] += 1

        # ---------------- turn3: R (64=k2, (k1 64, o2s 128)) ---------------
        R = sb.tile([KEEP, KEEP, P], BF16, tag="tagE", name=f"R_{b}")
        for kg in range(16):
            pt = ptp.tile([KEEP, 4, P], BF16, tag="tp", name=f"t3_{b}_{kg}")
            for i in range(4):
                nc.tensor.transpose(pt[:, i, :], O2[:, kg * 4 + i, :], ident[:])
            cp(R[:, kg * 4:kg * 4 + 4, :], pt[:])

        # ---------------- invW: G[wh] (128=w, (j 2, k1 64, c' 64)) ---------
        G = [sb.tile([P, 2, KEEP, BS], BF16, tag=f"tagAB{wh}", name=f"G{wh}_{b}")
             for wh in range(2)]
        for wh in range(2):
            for j in range(2):       # 0: Gre, 1: Gim
                for nn in range(8):  # 8 k1 per chunk
                    ps = pmm.tile([P, 8, BS], F32, tag="mm",
                                  name=f"psW_{b}_{wh}_{j}_{nn}")
                    nc.tensor.matmul(
                        ps[:], LIW[wh][j][0],
                        R[:, nn * 8:(nn + 1) * 8, 0:KEEP],
                        start=True, stop=False)
                    nc.tensor.matmul(
                        ps[:], LIW[wh][j][1],
                        R[:, nn * 8:(nn + 1) * 8, KEEP:P],
                        start=False, stop=True)
                    cp(G[wh][:, j, nn * 8:(nn + 1) * 8, :], ps[:])

        # ---------------- turn4: Ght (128=k1s, (w 256, c' 64)) -------------
        Ght = [sb.tile([P, P, BS], BF16, tag=f"tagCD{wh}", name=f"Ght{wh}_{b}")
               for wh in range(2)]
        for wh in range(2):
            for cg in range(16):
                pt = ptp.tile([P, 4, P], BF16, tag="tp", name=f"t4_{b}_{wh}_{cg}")
                for i in range(4):
                    # free slice (j 2, k1 64) -> out partitions [k1re | k1im]
                    nc.tensor.transpose(pt[:, i, :], G[wh][:, :, :, cg * 4 + i],
                                        ident[:])
                cp(Ght[wh][:, :, cg * 4:cg * 4 + 4], pt[:, :, :].rearrange(
                    "a b c -> a c b"))

        # ---------------- invH + residual + store --------------------------
        for hc in range(2):
            for wq in range(4):      # groups of 64 w -> 1MB stores
                ot = outp.tile([P, 64, BS], BF16, tag="ot",
                               name=f"ot_{b}_{hc}_{wq}")
                for nn in range(8):  # N=512 pieces (8 w each)
                    wg = wq * 8 + nn         # global 8-w group (0..31)
                    ps = pmm.tile([P, 8, BS], F32, tag="mm",
                                  name=f"psH_{b}_{hc}_{wg}")
                    nc.tensor.matmul(
                        ps[:], LIH[hc],
                        Ght[wg // 16][:, (wg % 16) * 8:(wg % 16) * 8 + 8, :],
                        start=True, stop=True)
                    nc.vector.tensor_tensor(
                        out=ot[:, nn * 8:(nn + 1) * 8, :], in0=ps[:],
                        in1=xt[hc][wq][:, nn * 8:(nn + 1) * 8, :],
                        op=mybir.AluOpType.add)
                if no_io:
                    nc.scalar.dma_start(out=out[b, 0:1, 0:1, :],
                                        in_=ot[0:1, 0:1, :])
                else:
                    nc.scalar.dma_start(
                        out=out[b, hc * P:(hc + 1) * P,
                                wq * 64:(wq + 1) * 64, :],
                        in_=ot[:])


def _prepare_in_maps(x, w1, b1, w2, b2):
    bf = ml_dtypes.bfloat16
    ffwd, lbw, liw, lih = _host_consts()
    x = np.asarray(x, dtype=np.float32)

    in_maps = []
    for n in range(NB):
        xs = np.ascontiguousarray(x[..., n * BS:(n + 1) * BS])
        w1n = np.asarray(w1[:, n], dtype=np.float32)   # (2,64,128)
        w2n = np.asarray(w2[:, n], dtype=np.float32)   # (2,128,64)
        b1n = np.asarray(b1[:, n], dtype=np.float32)   # (2,128)
        b2n = np.asarray(b2[:, n], dtype=np.float32)   # (2,64)
        m1 = np.stack([
            np.stack([w1n[0], -w1n[1]]),
            np.stack([w1n[1], w1n[0]]),
        ]).astype(bf)                                   # (2,2,64,128)
        m2 = np.stack([
            np.concatenate([w2n[0], w2n[1]], axis=1),
            np.concatenate([-w2n[1], w2n[0]], axis=1),
        ]).astype(bf)                                   # (2,128,128)
        in_maps.append({
            "xbf": xs.astype(bf),
            "ffwd": ffwd,
            "lbw": lbw,
            "m1": m1,
            "m2": m2,
            "b1s": b1n[:, :, None].copy(),
            "b2s": np.concatenate([b2n[0], b2n[1]])[:, None].copy(),
            "liw": liw,
            "lih": lih,
        })

    return in_maps


def kernel(x, w1, b1, w2, b2):
    global _CACHED_NC
    if _CACHED_NC is None:
        _CACHED_NC = _build_nc()
    nc = _CACHED_NC
    in_maps = _prepare_in_maps(x, w1, b1, w2, b2)
    res = run_bass_kernel_spmd(nc, in_maps, list(range(NB)))
    return np.concatenate(
        [res.results[i]["out"].astype(np.float32) for i in range(NB)], axis=-1)
